# revision 1
# baseline (speedup 1.0000x reference)
"""DecoderRNN Trainium2 kernel (8 NeuronCores).

Sharding: batch-parallel recurrence (16 rows/core), vocab-parallel output
projection (4000 cols/core). Hidden-state history exchanged via 2 AllGathers
(one overlapped with the recurrence); log-softmax normalizer via 1 AllReduce
with raw logits staged in DRAM between passes.

Self-contained: hardcodes all shapes from the problem spec.
"""
import math
from contextlib import ExitStack

import numpy as np
import ml_dtypes

import concourse.bacc as bacc
import concourse.bass as bass
import concourse.tile as tile
from concourse import mybir
from concourse.bass import AP
from concourse.masks import make_identity

F32 = mybir.dt.float32
BF16 = mybir.dt.bfloat16
I32 = mybir.dt.int32
AF = mybir.ActivationFunctionType

# problem constants
B, L, H, V, WORD, T = 128, 64, 512, 32000, 512, 32
NC = 8            # cores
BL = B // NC      # local batch rows = 16
NR = BL * L       # local attention rows = 1024
RK = NR // 128    # row chunks = 8
HK = H // 128     # h chunks = 4
TS = T - 1        # decode steps = 31
VL = V // NC      # local vocab = 4000
G3 = 3 * H        # 1536


def _mm(nc, out, lhsT, rhs, start, stop):
    nc.tensor.matmul(out, lhsT, rhs, start=start, stop=stop)


def build_program(t_steps=TS, n_cores=NC, no_collectives=False, no_phase2=False):
    """Builds the SPMD Bass program. Returns compiled nc."""
    nc = bacc.Bacc("TRN2", target_bir_lowering=False, debug=False,
                   num_devices=n_cores)
    rg = [list(range(n_cores))]
    bfull = n_cores * BL
    ag_split = min(16, t_steps)   # first AllGather covers t < ag_split

    # ---- I/O tensors (per-core data via in_maps) ----
    def din(name, shape, dt=F32):
        return nc.dram_tensor(name, shape, dt, kind="ExternalInput")

    enc_nat = din("enc_nat", [RK, 128, H])          # rows (b*64+l)
    encT = din("encT", [HK, 128, NR])
    hid0 = din("hid0", [BL, H])
    hidT0 = din("hidT0", [HK, 128, BL], BF16)
    tgt_idx = din("tgt_idx", [4, 128, 1], I32)      # rows t*16+b, padded 512
    embW = din("embW", [V, WORD])
    w1eT = din("w1eT", [HK, 128, H])
    w1hT = din("w1hT", [HK, 128, H], BF16)
    w2T = din("w2T", [HK, 128, H], BF16)
    w3T = din("w3T", [HK, 128, H], BF16)
    vT = din("vT", [HK, 128, 1], BF16)
    b1 = din("b1", [128, HK])
    b2 = din("b2", [128, HK])
    b3 = din("b3", [128, HK])
    wiheT = din("wiheT", [HK, 128, G3])
    wihcT = din("wihcT", [HK, 128, G3], BF16)
    whhT = din("whhT", [HK, 128, G3], BF16)
    bih = din("bih", [1, G3])
    bhh = din("bhh", [1, G3])
    outWT = din("outWT", [HK, 128, VL], BF16)
    outb = din("outb", [1, VL], BF16)
    out_lp = nc.dram_tensor("out_lp", [bfull, t_steps, VL], F32,
                            kind="ExternalOutput")

    with tile.TileContext(nc) as tc, ExitStack() as top:
        dram = top.enter_context(tc.tile_pool(name="dram", bufs=1, space="DRAM"))
        hist = dram.tile([t_steps, BL, H], BF16)
        gat1 = dram.tile([n_cores, ag_split, BL, H], BF16)
        gat2 = (dram.tile([n_cores, t_steps - ag_split, BL, H], BF16, name="gat2")
                if t_steps > ag_split else None)
        lstage = dram.tile([t_steps, bfull, VL], BF16)
        ar_in = dram.tile([bfull, t_steps], F32)
        ar_out = dram.tile([bfull, t_steps], F32)

        def gat_of(t):
            return (gat1, t) if t < ag_split else (gat2, t - ag_split)

        # ---------------- persistent SBUF (whole kernel) ----------------
        per = top.enter_context(tc.tile_pool(name="per", bufs=1))
        ident = per.tile([128, 128], F32)
        make_identity(nc, ident[:])
        ones32 = per.tile([1, 128], F32)
        nc.gpsimd.memset(ones32[:], 1.0)
        onesb = per.tile([1, 128], BF16)
        nc.gpsimd.memset(onesb[:], 1.0)
        identb = per.tile([128, 128], BF16)
        nc.vector.tensor_copy(identb[:], ident[:])
        sumexp = per.tile([max(bfull, 1), t_steps], F32)

        with ExitStack() as ph1:
            p1 = ph1.enter_context(tc.tile_pool(name="p1", bufs=1))
            # persistent phase-1 tensors
            enc_sb = p1.tile([128, RK, H + 1], F32)
            nc.sync.dma_start(enc_sb[:, :, 0:H],
                              enc_nat.ap().rearrange("k p h -> p k h"))
            nc.gpsimd.memset(enc_sb[:, :, H:H + 1], 1.0)
            w1hT_sb = p1.tile([128, HK, H], BF16)
            nc.sync.dma_start(w1hT_sb[:], w1hT.ap().rearrange("k p h -> p k h"))
            w2T_sb = p1.tile([128, HK, H], BF16)
            nc.sync.dma_start(w2T_sb[:], w2T.ap().rearrange("k p h -> p k h"))
            w3T_sb = p1.tile([128, HK, H], BF16)
            nc.sync.dma_start(w3T_sb[:], w3T.ap().rearrange("k p h -> p k h"))
            vT_sb = p1.tile([128, HK], BF16)
            nc.sync.dma_start(vT_sb[:], vT.ap().rearrange("k p one -> p (k one)"))
            b1_sb = p1.tile([128, HK], F32)
            nc.sync.dma_start(b1_sb[:], b1.ap())
            b2_sb = p1.tile([128, HK], F32)
            nc.sync.dma_start(b2_sb[:], b2.ap())
            b3_sb = p1.tile([128, HK], F32)
            nc.sync.dma_start(b3_sb[:], b3.ap())
            wihcT_sb = p1.tile([128, HK, G3], BF16)
            nc.sync.dma_start(wihcT_sb[:], wihcT.ap().rearrange("k p h -> p k h"))
            whhT_sb = p1.tile([128, HK, G3], BF16)
            nc.sync.dma_start(whhT_sb[:], whhT.ap().rearrange("k p h -> p k h"))
            bhh_sb = p1.tile([1, G3], F32)
            nc.sync.dma_start(bhh_sb[:], bhh.ap())
            encprojT = p1.tile([128, HK, BL, L], BF16)
            gi_emb = p1.tile([128, 4, G3], BF16)
            mask_sb = p1.tile([128, RK, BL], F32)
            nc.gpsimd.memset(mask_sb[:], 0.0)

            # pools for per-step working tiles
            hidp = ph1.enter_context(tc.tile_pool(name="hidp", bufs=2))
            wka = ph1.enter_context(tc.tile_pool(name="wka", bufs=1))
            wk = ph1.enter_context(tc.tile_pool(name="wk", bufs=2))
            gw = ph1.enter_context(tc.tile_pool(name="gw", bufs=1))
            # PSUM budget is 8 banks total, statically reserved per pool:
            # pd 3 (dense m-tiles) + pgg 3 (gh/gi/phase0) + pmisc 2 = 8
            pd = ph1.enter_context(tc.tile_pool(name="pd", bufs=3, space="PSUM"))
            pgg = ph1.enter_context(tc.tile_pool(name="pgg", bufs=1, space="PSUM"))
            pmisc = ph1.enter_context(tc.tile_pool(name="pmisc", bufs=1, space="PSUM"))

            # ---------------- phase 0: one-time precompute ----------------
            with ExitStack() as ph0:
                p0 = ph0.enter_context(tc.tile_pool(name="p0", bufs=1))
                p0s = ph0.enter_context(tc.tile_pool(name="p0s", bufs=2))
                w1eT_sb = p0.tile([128, HK, H], F32)
                nc.sync.dma_start(w1eT_sb[:], w1eT.ap().rearrange("k p h -> p k h"))
                bih_sb = p0.tile([1, G3], F32)
                nc.sync.dma_start(bih_sb[:], bih.ap())
                embT = p0.tile([128, HK, 4, 128], F32)
                with ExitStack() as ph00:
                    p00 = ph00.enter_context(tc.tile_pool(name="p00", bufs=1))
                    idx_sb = p00.tile([128, 4], I32)
                    nc.sync.dma_start(idx_sb[:],
                                      tgt_idx.ap().rearrange("r p one -> p (r one)"))
                    embg = p00.tile([128, 4, WORD], F32)
                    for r in range(4):
                        nc.gpsimd.indirect_dma_start(
                            out=embg[:, r, :], out_offset=None, in_=embW.ap(),
                            in_offset=bass.IndirectOffsetOnAxis(
                                ap=idx_sb[:, r:r + 1], axis=0))
                    # transpose embeddings: embT[p=h%128, k, r, rows128]
                    for r in range(4):
                        for k in range(HK):
                            pt = pgg.tile([128, 128], F32, tag="pgg")
                            nc.tensor.transpose(
                                pt[:], embg[:, r, k * 128:(k + 1) * 128], ident[:])
                            nc.vector.tensor_copy(embT[:, k, r, :], pt[:])
                # gi_emb[p=row%128, r, f] = emb @ Wih_e.T + bih  (stream Wih_e)
                for r in range(4):
                    pge = pgg.tile([128, G3], F32, tag="pgg")
                    for k in range(HK):
                        wch = p0s.tile([128, G3], F32, tag="wch")
                        nc.sync.dma_start(wch[:], wiheT.ap()[k])
                        for j in range(3):
                            _mm(nc, pge[:, j * 512:(j + 1) * 512], embT[:, k, r, :],
                                wch[:, j * 512:(j + 1) * 512], k == 0, False)
                    for j in range(3):
                        _mm(nc, pge[:, j * 512:(j + 1) * 512], ones32[:],
                            bih_sb[:, j * 512:(j + 1) * 512], False, True)
                    nc.vector.tensor_copy(gi_emb[:, r, :], pge[:])
                # encprojT[p=h'%128, m, b, l] = W1e @ enc.T  (stream enc.T)
                for m in range(HK):
                    pep = pgg.tile([128, NR], F32, tag="pgg")
                    for k in range(HK):
                        ech = p0s.tile([128, NR], F32, tag="ech")
                        nc.sync.dma_start(ech[:], encT.ap()[k])
                        for j in range(2):
                            _mm(nc, pep[:, j * 512:(j + 1) * 512],
                                w1eT_sb[:, k, m * 128:(m + 1) * 128],
                                ech[:, j * 512:(j + 1) * 512], k == 0, k == HK - 1)
                    nc.vector.tensor_copy(
                        encprojT[:, m, :, :],
                        pep[:].rearrange("p (b l) -> p b l", b=BL))

            # ---------------- phase 1: recurrence ----------------
            hid = hidp.tile([BL, H], F32, tag="hid")
            nc.sync.dma_start(hid[:], hid0.ap())
            hidT = hidp.tile([128, HK, BL], BF16, tag="hidT")
            nc.sync.dma_start(hidT[:], hidT0.ap().rearrange("k p b -> p k b"))

            for t in range(t_steps):
                # gh = Whh @ hid + bhh -> evacuated to SBUF (psum shared w/ gi)
                pgh = pgg.tile([BL, G3], F32, tag="pgg")
                for k in range(HK):
                    for j in range(3):
                        _mm(nc, pgh[:, j * 512:(j + 1) * 512], hidT[:, k, :],
                            whhT_sb[:, k, j * 512:(j + 1) * 512], k == 0, False)
                for j in range(3):
                    _mm(nc, pgh[:, j * 512:(j + 1) * 512], ones32[:, 0:BL],
                        bhh_sb[:, j * 512:(j + 1) * 512], False, True)
                gh_sb = gw.tile([BL, G3], F32, tag="gh_sb")
                nc.vector.tensor_copy(gh_sb[:], pgh[:])

                # hidproj = W1h @ hid
                php = pmisc.tile([128, HK, BL], F32, tag="pmisc")
                for m in range(HK):
                    for k in range(HK):
                        _mm(nc, php[:, m, :], w1hT_sb[:, k, m * 128:(m + 1) * 128],
                            hidT[:, k, :], k == 0, k == HK - 1)

                # a1 = tanh(encproj + hidproj + b1)  [h-part layout]
                a1T = wka.tile([128, HK, NR], BF16, tag="a1T")
                for m in range(HK):
                    pre = wk.tile([128, BL, L], F32, tag="a1pre")
                    hb = php[:, m, :]
                    hb = AP(tensor=hb.tensor, offset=hb.offset, ap=hb.ap + [[0, L]])
                    nc.vector.tensor_add(pre[:], encprojT[:, m, :, :], hb)
                    nc.scalar.activation(
                        out=a1T[:, m, :].rearrange("p (b l) -> p b l", b=BL),
                        in_=pre[:], func=AF.Tanh, bias=b1_sb[:, m:m + 1], scale=1.0)

                # dense2 / dense3 with tanh, half-split for psum
                # a3T reuses a1T's slot (a1 dead once dense2 is done)
                a2T = wka.tile([128, HK, NR], BF16, tag="a2T")
                a3T = wka.tile([128, HK, NR], BF16, tag="a1T")
                for (src, dst, wT, bias) in ((a1T, a2T, w2T_sb, b2_sb),
                                             (a2T, a3T, w3T_sb, b3_sb)):
                    for hf in range(2):
                        sl = slice(hf * 512, (hf + 1) * 512)
                        for m in range(HK):
                            pdt = pd.tile([128, 512], F32, tag="pd")
                            for k in range(HK):
                                _mm(nc, pdt[:], wT[:, k, m * 128:(m + 1) * 128],
                                    src[:, k, sl], k == 0, k == HK - 1)
                            nc.scalar.activation(out=dst[:, m, sl], in_=pdt[:],
                                                 func=AF.Tanh,
                                                 bias=bias[:, m:m + 1], scale=1.0)

                # eT[p=row%128, m] = a3 . v ; exp
                pe = pmisc.tile([128, RK], F32, tag="pmisc")
                for m in range(RK):
                    for k in range(HK):
                        _mm(nc, pe[:, m:m + 1], a3T[:, k, m * 128:(m + 1) * 128],
                            vT_sb[:, k:k + 1], k == 0, k == HK - 1)
                expeT = gw.tile([128, RK], F32, tag="expeT")
                nc.scalar.activation(out=expeT[:], in_=pe[:], func=AF.Exp)

                # mask strips (zeros persist from phase 0)
                for k in range(RK):
                    nc.vector.tensor_copy(mask_sb[0:64, k, 2 * k:2 * k + 1],
                                          expeT[0:64, k:k + 1])
                    nc.vector.tensor_copy(mask_sb[64:128, k, 2 * k + 1:2 * k + 2],
                                          expeT[64:128, k:k + 1])

                # ctxu[b, h] (+ Z in col H) = mask.T @ [enc | 1]
                pcu = pmisc.tile([BL, H + 1], F32, tag="pmisc")
                for k in range(RK):
                    _mm(nc, pcu[:, 0:H], mask_sb[:, k, :], enc_sb[:, k, 0:H],
                        k == 0, k == RK - 1)
                    _mm(nc, pcu[:, H:H + 1], mask_sb[:, k, :], enc_sb[:, k, H:H + 1],
                        k == 0, k == RK - 1)
                rcpZ = gw.tile([BL, 1], F32, tag="rcpZ")
                nc.vector.reciprocal(rcpZ[:], pcu[:, H:H + 1])
                ctxu = gw.tile([BL, H], F32, tag="ctxu")
                nc.vector.tensor_copy(ctxu[:], pcu[:, 0:H])
                diag = gw.tile([BL, BL], F32, tag="diag")
                nc.vector.tensor_scalar_mul(diag[:], ident[0:BL, 0:BL], rcpZ[:])

                # ctxT[h, b] = ctxu.T scaled by rcpZ (transpose+scale via diag mm)
                pct = pmisc.tile([128, HK, BL], F32, tag="pmisc")
                for m in range(HK):
                    _mm(nc, pct[:, m, :], ctxu[:, m * 128:(m + 1) * 128], diag[:],
                        True, True)
                ctxT = gw.tile([128, HK, BL], BF16, tag="ctxT")
                nc.vector.tensor_copy(ctxT[:], pct[:])

                # gi_ctx = Wih_c @ ctx
                pgi = pgg.tile([BL, G3], F32, tag="pgg")
                for k in range(HK):
                    for j in range(3):
                        _mm(nc, pgi[:, j * 512:(j + 1) * 512], ctxT[:, k, :],
                            wihcT_sb[:, k, j * 512:(j + 1) * 512], k == 0, k == HK - 1)

                # gates (stage this step's gi_emb rows to partitions 0:16 via DMA)
                po = (t % 8) * BL
                tc_ = t // 8
                ge_t = wk.tile([BL, G3], BF16, tag="ge_t")
                nc.sync.dma_start(ge_t[:], gi_emb[po:po + BL, tc_, :])
                rz = gw.tile([BL, 2 * H], F32, tag="rz")
                nc.vector.tensor_add(rz[:], pgi[:, 0:2 * H], gh_sb[:, 0:2 * H])
                nc.vector.tensor_add(rz[:], rz[:], ge_t[:, 0:2 * H])
                nc.scalar.activation(out=rz[:], in_=rz[:], func=AF.Sigmoid)
                n1 = gw.tile([BL, H], F32, tag="n1")
                nc.vector.tensor_add(n1[:], pgi[:, 2 * H:G3], ge_t[:, 2 * H:G3])
                n2 = gw.tile([BL, H], F32, tag="n2")
                nc.vector.tensor_mul(n2[:], rz[:, 0:H], gh_sb[:, 2 * H:G3])
                nc.vector.tensor_add(n1[:], n1[:], n2[:])
                nc.scalar.activation(out=n1[:], in_=n1[:], func=AF.Tanh)
                nc.vector.tensor_sub(n2[:], hid[:], n1[:])          # d = hid - n
                nc.vector.tensor_mul(n2[:], rz[:, H:2 * H], n2[:])  # z*d
                hid = hidp.tile([BL, H], F32, tag="hid")
                nc.vector.tensor_add(hid[:], n1[:], n2[:])

                # hidT for next step's matmuls; hid bf16 row-layout for history
                pht = pmisc.tile([128, HK, BL], F32, tag="pmisc")
                for k in range(HK):
                    nc.tensor.transpose(pht[:, k, :], hid[:, k * 128:(k + 1) * 128],
                                        ident[0:BL, 0:BL])
                hidT = hidp.tile([128, HK, BL], BF16, tag="hidT")
                nc.vector.tensor_copy(hidT[:], pht[:])
                hidb = hidp.tile([BL, H], BF16, tag="hidb")
                nc.vector.tensor_copy(hidb[:], hid[:])
                nc.sync.dma_start(hist[t], hidb[:])

                if not no_collectives and t == ag_split - 1:
                    nc.gpsimd.collective_compute(
                        "AllGather", mybir.AluOpType.bypass, replica_groups=rg,
                        ins=[hist[0:ag_split].opt()], outs=[gat1[:].opt()])
                if not no_collectives and gat2 is not None and t == t_steps - 1:
                    nc.gpsimd.collective_compute(
                        "AllGather", mybir.AluOpType.bypass, replica_groups=rg,
                        ins=[hist[ag_split:t_steps].opt()], outs=[gat2[:].opt()])

        # ---------------- phase 2: output projection + log-softmax ----------
        if not no_phase2:
            with ExitStack() as ph2:
                p2 = ph2.enter_context(tc.tile_pool(name="p2", bufs=1))
                outWT_sb = p2.tile([128, HK, VL], BF16)
                nc.sync.dma_start(outWT_sb[:], outWT.ap().rearrange("k p v -> p k v"))
                outb_sb = p2.tile([1, VL], BF16)
                nc.sync.dma_start(outb_sb[:], outb.ap())
                w2p = ph2.enter_context(tc.tile_pool(name="w2p", bufs=3))
                pl = ph2.enter_context(tc.tile_pool(name="pl", bufs=3, space="PSUM"))
                pt2 = ph2.enter_context(tc.tile_pool(name="pt2", bufs=2, space="PSUM"))

                # quarter column ranges (512-aligned for psum banks)
                quarters = []
                for q in range(4):
                    c0 = q * 1024
                    c1 = min(c0 + 1024, VL)
                    quarters.append((c0, c1))

                # pass A: logits -> lstage (bf16) + sumexp partials
                for t in range(t_steps):
                    gat, tt = gat_of(t)
                    hfull = w2p.tile([bfull, H], BF16, tag="hfull")
                    nc.sync.dma_start(hfull[:], gat[:, tt, :, :])
                    hT = w2p.tile([128, HK, bfull], BF16, tag="hT")
                    for k in range(HK):
                        ptr = pt2.tile([128, bfull], BF16, tag="ptr")
                        nc.tensor.transpose(ptr[:], hfull[:, k * 128:(k + 1) * 128],
                                            identb[0:bfull, 0:bfull])
                        nc.vector.tensor_copy(hT[:, k, :], ptr[:])
                    lgt = w2p.tile([bfull, 4096], BF16, tag="lgt")
                    ses = w2p.tile([bfull, 4], F32, tag="ses")
                    for q, (c0, c1) in enumerate(quarters):
                        w = c1 - c0
                        plg = pl.tile([bfull, 1024], F32, tag="plg")
                        for k in range(HK):
                            for cc in range(c0, c1, 512):
                                ce = min(cc + 512, c1)
                                _mm(nc, plg[:, cc - c0:ce - c0], hT[:, k, :],
                                    outWT_sb[:, k, cc:ce], k == 0, False)
                        for cc in range(c0, c1, 512):
                            ce = min(cc + 512, c1)
                            _mm(nc, plg[:, cc - c0:ce - c0], onesb[:, 0:bfull],
                                outb_sb[:, cc:ce], False, True)
                        exps = w2p.tile([bfull, 1024], BF16, tag="exps")
                        nc.scalar.activation(out=exps[:, 0:w], in_=plg[:, 0:w],
                                             func=AF.Exp, accum_out=ses[:, q:q + 1])
                        nc.vector.tensor_copy(lgt[:, c0:c0 + w], plg[:, 0:w])
                    nc.vector.reduce_sum(out=sumexp[:, t:t + 1],
                                         in_=ses[:].rearrange("p (x q) -> p x q", x=1),
                                         axis=mybir.AxisListType.X)
                    nc.sync.dma_start(lstage[t], lgt[:, 0:VL])

                # exchange sumexp partials (single AllReduce)
                nc.sync.dma_start(ar_in[:], sumexp[:])
                if not no_collectives:
                    nc.gpsimd.collective_compute(
                        "AllReduce", mybir.AluOpType.add, replica_groups=rg,
                        ins=[ar_in[:].opt()], outs=[ar_out[:].opt()])
                gse = w2p.tile([bfull, t_steps], F32, tag="gse")
                nc.sync.dma_start(gse[:], ar_out[:])
                nlz = w2p.tile([bfull, t_steps], F32, tag="nlz")
                nc.scalar.activation(out=nlz[:], in_=gse[:], func=AF.Ln)
                nc.vector.tensor_scalar_mul(nlz[:], nlz[:], -1.0)

                # pass B: logp = logits - logZ -> out
                for t in range(t_steps):
                    lg = w2p.tile([bfull, VL], BF16, tag="lg")
                    nc.sync.dma_start(lg[:], lstage[t])
                    lp = w2p.tile([bfull, VL], F32, tag="lp")
                    nc.vector.tensor_scalar_add(lp[:], lg[:], nlz[:, t:t + 1])
                    nc.sync.dma_start(out_lp.ap()[:, t, :], lp[:])

    nc.compile()
    return nc


_NC_CACHE = {}


def _get_program(t_steps=TS, n_cores=NC, **kw):
    key = (t_steps, n_cores, tuple(sorted(kw.items())))
    if key not in _NC_CACHE:
        _NC_CACHE[key] = build_program(t_steps, n_cores, **kw)
    return _NC_CACHE[key]


def make_in_maps(inputs, t_steps=TS, n_cores=NC):
    """Host-side shard/layout prep. Pure data movement + dtype casts."""
    enc = np.asarray(inputs["encoder_outputs"], np.float32)
    ehid = np.asarray(inputs["encoder_hidden"], np.float32)
    targets = np.asarray(inputs["targets"])
    embW = np.ascontiguousarray(np.asarray(inputs["embed_W"], np.float32))
    aW1 = np.asarray(inputs["att_W1"], np.float32)
    aW2 = np.asarray(inputs["att_W2"], np.float32)
    aW3 = np.asarray(inputs["att_W3"], np.float32)
    ab1 = np.asarray(inputs["att_b1"], np.float32)
    ab2 = np.asarray(inputs["att_b2"], np.float32)
    ab3 = np.asarray(inputs["att_b3"], np.float32)
    av = np.asarray(inputs["att_v"], np.float32)
    gWih = np.asarray(inputs["gru_Wih"], np.float32)
    gWhh = np.asarray(inputs["gru_Whh"], np.float32)
    gbih = np.asarray(inputs["gru_bih"], np.float32)
    gbhh = np.asarray(inputs["gru_bhh"], np.float32)
    oW = np.asarray(inputs["out_W"], np.float32)
    ob = np.asarray(inputs["out_b"], np.float32)

    def chunkT(w, dt=np.float32):  # (out,in)->(in,out) h-chunked: (HK,128,out)
        wt = np.ascontiguousarray(w.T.astype(dt))
        return wt.reshape(HK, 128, w.shape[0])

    bf = ml_dtypes.bfloat16
    shared = {
        "embW": embW,
        "w1eT": chunkT(aW1[:, :H]),
        "w1hT": chunkT(aW1[:, H:], bf),
        "w2T": chunkT(aW2, bf), "w3T": chunkT(aW3, bf),
        "vT": np.ascontiguousarray(av[0].astype(bf)).reshape(HK, 128, 1),
        "b1": np.ascontiguousarray(ab1.reshape(HK, 128).T),
        "b2": np.ascontiguousarray(ab2.reshape(HK, 128).T),
        "b3": np.ascontiguousarray(ab3.reshape(HK, 128).T),
        "wiheT": chunkT(gWih[:, :WORD]),
        "wihcT": chunkT(gWih[:, WORD:], bf),
        "whhT": chunkT(gWhh, bf),
        "bih": gbih.reshape(1, G3).astype(np.float32),
        "bhh": gbhh.reshape(1, G3).astype(np.float32),
    }
    in_maps = []
    for c in range(n_cores):
        bl0 = c * BL
        enc_l = enc[bl0:bl0 + BL].reshape(NR, H)
        idx = np.zeros(512, np.int32)
        idx[: BL * t_steps] = targets[bl0:bl0 + BL, :t_steps].T.astype(np.int32).ravel()
        m = dict(shared)
        m["enc_nat"] = np.ascontiguousarray(enc_l.reshape(RK, 128, H))
        m["encT"] = np.ascontiguousarray(enc_l.T).reshape(HK, 128, NR)
        m["hid0"] = np.ascontiguousarray(ehid[0, bl0:bl0 + BL])
        m["hidT0"] = np.ascontiguousarray(
            ehid[0, bl0:bl0 + BL].T.astype(bf)).reshape(HK, 128, BL)
        m["tgt_idx"] = idx.reshape(4, 128, 1)
        m["outWT"] = np.ascontiguousarray(
            oW[c * VL:(c + 1) * VL].T.astype(bf)).reshape(HK, 128, VL)
        m["outb"] = ob[c * VL:(c + 1) * VL].reshape(1, VL).astype(bf)
        in_maps.append(m)
    return in_maps


def run(inputs, trace=False, **trace_kw):
    from concourse import bass_utils
    nc = _get_program()
    in_maps = make_in_maps(inputs)
    res = bass_utils.run_bass_kernel_spmd(nc, in_maps, core_ids=list(range(NC)),
                                          trace=trace, **trace_kw)
    out = np.concatenate([res.results[c]["out_lp"] for c in range(NC)], axis=2)
    return out, res


def kernel(**inputs):
    return run(inputs)[0]



# revision 3
# speedup vs baseline: 1.3583x; 1.3583x over previous
"""DecoderRNN Trainium2 kernel v2 (8 NeuronCores).

Restructured from the baseline for overlap:
- Fused phases: vocab-parallel output projection (pass A) and log-softmax
  finalize (pass B) interleave into the recurrence as pump slots.
- Chunked hidden-state AllGathers + chunked sumexp AllGathers.
- fp8 (e4m3) DoubleRow matmuls for attention dense layers, GRU gate
  matvecs, and the output projection. Weights pre-scaled x64 on host to
  avoid the fp8 subnormal zone; rescaled at psum evacuation.
- Transposed GRU gate layout [h-part, batch-free]: 128-partition gate
  arithmetic, no per-step hidden-state transposes.
- Sigmoid via tanh identity (sigmoid(x) = .5 + .5 tanh(x/2)) so the whole
  recurrence stays on one activation table (tanh+exp).
- Batch split in halves; the two independent recurrences interleave to
  fill engine bubbles of the serial chain.

Self-contained: hardcodes all shapes from the problem spec.
"""
import numpy as np
import ml_dtypes
from contextlib import ExitStack

import concourse.bacc as bacc
import concourse.bass as bass
import concourse.tile as tile
from concourse import mybir
from concourse.bass import AP
from concourse.masks import make_identity

F32 = mybir.dt.float32
BF16 = mybir.dt.bfloat16
FP8 = mybir.dt.float8e4
I32 = mybir.dt.int32
AF = mybir.ActivationFunctionType
DR = mybir.MatmulPerfMode.DoubleRow
ALU = mybir.AluOpType

# problem constants
B, L, H, V, WORD, T = 128, 64, 512, 32000, 512, 32
NC = 8            # cores
BL = B // NC      # local batch rows = 16
NR = BL * L       # local attention rows = 1024
RK = NR // 128    # row chunks = 8
HK = H // 128     # h chunks = 4
TS = T - 1        # decode steps = 31
VL = V // NC      # local vocab = 4000
G3 = 3 * H        # 1536
NH = 2            # batch halves per core
BH = BL // NH     # batch rows per half = 8
RH = RK // NH     # row chunks per half = 4
WS = 64.0         # fp8 weight scale
RS = 1.0 / WS

# phase-2 chunk schedule: [start, end) step ranges
CHUNKS = [(0, 8), (8, 16), (16, 22), (22, 26), (26, 29), (29, 31)]


def _mm(nc, out, lhsT, rhs, start, stop, pm=None):
    nc.tensor.matmul(out, lhsT, rhs, start=start, stop=stop, perf_mode=pm)


def build_program(t_steps=TS, n_cores=NC, no_collectives=False):
    nc = bacc.Bacc("TRN2", target_bir_lowering=False, debug=False,
                   num_devices=n_cores)
    rg = [list(range(n_cores))]
    bfull = n_cores * BL
    chunks = [(a, min(b, t_steps)) for a, b in CHUNKS if a < t_steps]

    def din(name, shape, dt=F32):
        return nc.dram_tensor(name, shape, dt, kind="ExternalInput")

    # ---- inputs (host-prepped layouts; *64 = values pre-scaled by WS) ----
    enc_in = din("enc_in", [RK, 128, H], BF16)        # natural rows b*64+l
    encT_in = din("encT_in", [HK, 128, NR], BF16)     # enc transposed
    hid0T = din("hid0T", [HK, 128, BL])               # f32 transposed h0
    hid0T8 = din("hid0T8", [HK, 128, BL], FP8)
    tgt_idx = din("tgt_idx", [4, 128, 1], I32)        # rows t*16+b, pad 512
    embW = din("embW", [V, WORD])                     # f32 gather source
    w1eT_in = din("w1eT_in", [HK, 128, H], BF16)
    w1hT_in = din("w1hT_in", [HK, 128, H], FP8)       # *64
    w2T_in = din("w2T_in", [HK, 128, H], FP8)         # *64
    w3T_in = din("w3T_in", [HK, 128, H], FP8)         # *64
    vT_in = din("vT_in", [HK, 128, 1], FP8)           # *64
    b1T_in = din("b1T_in", [1, H], BF16)              # *64
    b2T_in = din("b2T_in", [1, H], BF16)              # *64
    b3T_in = din("b3T_in", [1, H], BF16)              # *64
    wiheT_in = din("wiheT_in", [HK, 128, G3], BF16)   # *64
    wihcT_in = din("wihcT_in", [HK, 128, G3], FP8)    # *64
    whhT_in = din("whhT_in", [HK, 128, G3], FP8)      # *64
    bihT_in = din("bihT_in", [1, G3], BF16)           # *64
    bhhT_in = din("bhhT_in", [1, G3], BF16)           # *64
    outWT_in = din("outWT_in", [HK, 128, VL], FP8)    # *64
    outb_in = din("outb_in", [1, VL], BF16)           # *64
    out_lp = nc.dram_tensor("out_lp", [bfull, t_steps, VL], F32,
                            kind="ExternalOutput")

    with tile.TileContext(nc) as tc, ExitStack() as top:
        dram = top.enter_context(tc.tile_pool(name="dram", bufs=1, space="DRAM"))
        hist = dram.tile([t_steps, HK, 128, BL], FP8)       # hidT history
        gat_as = "Local" if no_collectives else "Shared"
        gats = [dram.tile([n_cores, b - a, HK, 128, BL], FP8, name=f"gat{i}",
                          addr_space=gat_as)
                for i, (a, b) in enumerate(chunks)]
        sxins = [dram.tile([128, b - a], F32, name=f"sxin{i}")
                 for i, (a, b) in enumerate(chunks)]
        sxouts = [dram.tile([n_cores, 128, b - a], F32, name=f"sxout{i}",
                            addr_space="Shared")
                  for i, (a, b) in enumerate(chunks)]
        lstage = dram.tile([t_steps, bfull, VL], BF16)

        # ---------------- persistent SBUF ----------------
        per = top.enter_context(tc.tile_pool(name="per", bufs=1))
        ident = per.tile([128, 128], F32)
        make_identity(nc, ident[:])
        identb = per.tile([128, 128], BF16)
        nc.vector.tensor_copy(identb[:], ident[:])
        onesb = per.tile([1, 512], BF16)
        nc.gpsimd.memset(onesb[:], 1.0)
        onesc = per.tile([128, 1], BF16)
        nc.gpsimd.memset(onesc[:], 1.0)

        enc_sb = per.tile([128, RK, H], BF16)
        nc.sync.dma_start(enc_sb[:], enc_in.ap().rearrange("k p h -> p k h"))
        encprojT = per.tile([128, HK, BL, L], BF16)
        w1hT_sb = per.tile([128, HK, H], FP8)
        nc.sync.dma_start(w1hT_sb[:], w1hT_in.ap().rearrange("k p h -> p k h"))
        w2T_sb = per.tile([128, HK, H], FP8)
        nc.sync.dma_start(w2T_sb[:], w2T_in.ap().rearrange("k p h -> p k h"))
        w3T_sb = per.tile([128, HK, H], FP8)
        nc.sync.dma_start(w3T_sb[:], w3T_in.ap().rearrange("k p h -> p k h"))
        vT_sb = per.tile([128, HK, 1], FP8)
        nc.sync.dma_start(vT_sb[:], vT_in.ap().rearrange("k p o -> p k o"))
        b1T_sb = per.tile([1, H], BF16)
        nc.sync.dma_start(b1T_sb[:], b1T_in.ap())
        b2T_sb = per.tile([1, H], BF16)
        nc.sync.dma_start(b2T_sb[:], b2T_in.ap())
        b3T_sb = per.tile([1, H], BF16)
        nc.sync.dma_start(b3T_sb[:], b3T_in.ap())
        wihcT_sb = per.tile([128, HK, G3], FP8)
        nc.sync.dma_start(wihcT_sb[:], wihcT_in.ap().rearrange("k p h -> p k h"))
        whhT_sb = per.tile([128, HK, G3], FP8)
        nc.sync.dma_start(whhT_sb[:], whhT_in.ap().rearrange("k p h -> p k h"))
        bihT_sb = per.tile([1, G3], BF16)
        nc.sync.dma_start(bihT_sb[:], bihT_in.ap())
        bhhT_sb = per.tile([1, G3], BF16)
        nc.sync.dma_start(bhhT_sb[:], bhhT_in.ap())
        outWT_sb = per.tile([128, HK, VL], FP8)
        nc.sync.dma_start(outWT_sb[:], outWT_in.ap().rearrange("k p v -> p k v"))
        outb_sb = per.tile([1, VL], BF16)
        nc.sync.dma_start(outb_sb[:], outb_in.ap())
        giT_emb = per.tile([128, 12, 4, 128], BF16)         # *64, transposed
        masks = [per.tile([128, RH, BH], BF16, name=f"mask{h}")
                 for h in range(NH)]
        for h in range(NH):
            nc.gpsimd.memset(masks[h][:], 0.0)
        sumexp = per.tile([128, t_steps], F32)
        nlz = per.tile([128, t_steps], F32)                 # -log Z

        # ---------------- psum pools (8 banks exactly) ----------------
        # pd 2x2 banks, pl 2x1; pg+pm opened after phase 0 (ptb scoped there)
        pd = top.enter_context(tc.tile_pool(name="pd", bufs=4, space="PSUM"))
        pl = top.enter_context(tc.tile_pool(name="pl", bufs=2, space="PSUM"))

        # pools for per-step tiles
        hidp = top.enter_context(tc.tile_pool(name="hidp", bufs=2))
        wk = top.enter_context(tc.tile_pool(name="wk", bufs=2))
        ap_ = top.enter_context(tc.tile_pool(name="ap", bufs=2))

        # ---------------- phase 0 ----------------
        with ExitStack() as ph0:
            p0 = ph0.enter_context(tc.tile_pool(name="p0", bufs=1))
            ptb = ph0.enter_context(tc.tile_pool(name="ptb", bufs=1,
                                                 space="PSUM"))
            encT_sb = p0.tile([128, HK, NR], BF16)
            nc.sync.dma_start(encT_sb[:], encT_in.ap().rearrange("k p r -> p k r"))
            w1eT_sb = p0.tile([128, HK, H], BF16)
            nc.sync.dma_start(w1eT_sb[:], w1eT_in.ap().rearrange("k p h -> p k h"))
            wiheT_sb = p0.tile([128, HK, G3], BF16)
            nc.sync.dma_start(wiheT_sb[:], wiheT_in.ap().rearrange("k p h -> p k h"))
            idx_sb = p0.tile([128, 4], I32)
            nc.sync.dma_start(idx_sb[:], tgt_idx.ap().rearrange("r p o -> p (r o)"))
            embT = p0.tile([128, HK, 4, 128], BF16)

            # embedding gather + transpose + gi_emb  (chunk r covers steps
            # 8r..8r+7, so r=0 first unblocks step 0 quickly)
            for r in range(4):
                embg = p0.tile([128, WORD], F32, tag="embg", name="embg")
                nc.gpsimd.indirect_dma_start(
                    out=embg[:], out_offset=None, in_=embW.ap(),
                    in_offset=bass.IndirectOffsetOnAxis(ap=idx_sb[:, r:r + 1],
                                                        axis=0))
                embgb = p0.tile([128, WORD], BF16, tag="embgb", name="embgb")
                nc.vector.tensor_copy(embgb[:], embg[:])
                for k in range(HK):
                    ptr = ptb.tile([128, 128], BF16, tag="ptb")
                    nc.tensor.transpose(ptr[:], embgb[:, k * 128:(k + 1) * 128],
                                        identb[:])
                    nc.vector.tensor_copy(embT[:, k, r, :], ptr[:])
                # gi_emb rows = emb @ WihE.T (*64), then transpose chunks
                for j in range(3):
                    pge = pd.tile([128, 512], F32, tag="pd")
                    for k in range(HK):
                        _mm(nc, pge[:], embT[:, k, r, :],
                            wiheT_sb[:, k, j * 512:(j + 1) * 512],
                            k == 0, k == HK - 1)
                    ger = p0.tile([128, 512], BF16, tag="ger", name="ger")
                    nc.scalar.activation(out=ger[:], in_=pge[:],
                                         func=AF.Copy)
                    for cc in range(4):
                        ptg = ptb.tile([128, 128], BF16, tag="ptb")
                        nc.tensor.transpose(ptg[:],
                                            ger[:, cc * 128:(cc + 1) * 128],
                                            identb[:])
                        nc.vector.tensor_copy(giT_emb[:, 4 * j + cc, r, :],
                                              ptg[:])

            # encprojT[p,m,b,l] = W1e @ enc.T  (true scale, bf16)
            for m in range(HK):
                for j in range(2):
                    pep = pl.tile([128, 512], F32, tag="pl")
                    for k in range(HK):
                        _mm(nc, pep[:],
                            w1eT_sb[:, k, m * 128:(m + 1) * 128],
                            encT_sb[:, k, j * 512:(j + 1) * 512],
                            k == 0, k == HK - 1)
                    nc.scalar.activation(
                        out=encprojT[:, m].rearrange("p b l -> p (b l)")[
                            :, j * 512:(j + 1) * 512],
                        in_=pep[:], func=AF.Copy, scale=WS)

        pg = top.enter_context(tc.tile_pool(name="pg", bufs=1, space="PSUM"))
        pm_ = top.enter_context(tc.tile_pool(name="pm", bufs=1, space="PSUM"))

        # initial hidden state
        hidF = hidp.tile([128, HK, BL], F32, tag="hidF")
        nc.sync.dma_start(hidF[:], hid0T.ap().rearrange("k p b -> p k b"))
        hid8 = hidp.tile([128, HK, BL], FP8, tag="hid8")
        nc.sync.dma_start(hid8[:], hid0T8.ap().rearrange("k p b -> p k b"))

        # gates bank: pgt [0:384] | pe1 [384:388] | z1 [388]
        pgb = pg.tile([128, 512], F32)
        pgt = pgb[:, 0:384].rearrange("p (h i c b) -> p h i c b", h=NH, i=2, c=12)
        pgts = [[pgt[:, h, i] for i in range(2)] for h in range(NH)]
        # misc bank: pe0 [0:4] | z0 [4] | ctxT0 [8:40] | ctxT1 [40:72]
        #            php0 [80:112] | php1 [112:144]
        pms = pm_.tile([128, 160], F32)
        phps = [pms[:, 80 + h * 32:80 + (h + 1) * 32].rearrange(
                    "p (m b) -> p m b", m=HK) for h in range(NH)]
        zs = [pms[:, 4:5], pgb[:, 388:389]]

        def pe_of(h):
            return pms[:, 0:4] if h == 0 else pgb[:, 384:388]

        def ctxT_of(h):
            return pms[:, 8 + h * 32:8 + (h + 1) * 32].rearrange(
                "p (m b) -> p m b", m=HK)



        # ---------------- phase-2 work pump ----------------
        # Two queues: qA holds pass-A chunk pieces (PE+Act+DVE work) injected
        # at natural PE bubbles inside the step; qB holds DMA/Pool pieces
        # (hT/lgB loads, pass-B finalize, sumexp gathers) injected at step
        # boundaries.
        qA, qB = [], []

        def pump(q, n):
            for _ in range(min(n, len(q))):
                q.pop(0)()

        stash = {}

        def emit_hT_load(q):
            ci = next(i for i, (a, b) in enumerate(chunks) if a <= q < b)
            a, _ = chunks[ci]
            hT8 = wk.tile([128, HK, bfull], FP8, tag="hT8", name=f"hT8_{q}")
            for k in range(HK):
                nc.sync.dma_start(
                    hT8[:, k, :].rearrange("p (c b) -> p c b", c=n_cores),
                    gats[ci][:, q - a, k].rearrange("c p b -> p c b"))
            stash[("h", q)] = hT8

        def emit_passA_chunk(q, cc, lgt, ses):
            hT8 = stash[("h", q)]
            w = min(512, VL - cc)
            plg = pl.tile([128, 512], F32, tag="pl")
            for kp in range(0, HK, 2):
                _mm(nc, plg[:, 0:w], hT8[:, kp:kp + 2, :],
                    outWT_sb[:, kp:kp + 2, cc:cc + w], kp == 0, False, pm=DR)
            _mm(nc, plg[:, 0:w], onesb[0:1, 0:128],
                outb_sb[:, cc:cc + w], False, True)
            disc = wk.tile([128, 512], BF16, tag="disc", bufs=1)
            nc.scalar.activation(out=disc[:, 0:w], in_=plg[:, 0:w],
                                 func=AF.Exp, scale=RS,
                                 accum_out=ses[:, cc // 512:cc // 512 + 1])
            nc.vector.tensor_scalar_mul(lgt[:, cc:cc + w], plg[:, 0:w], RS)

        def emit_passA_fin(q, lgt, ses):
            nc.vector.reduce_sum(
                out=sumexp[:, q:q + 1],
                in_=ses[:].rearrange("p (x q) -> p x q", x=1),
                axis=mybir.AxisListType.X)
            nc.sync.dma_start(lstage[q], lgt[:, 0:VL])
            del stash[("h", q)]
            ci = chunk_of(q)
            passA_done_chunk[ci] += 1
            if passA_done_chunk[ci] == chunks[ci][1] - chunks[ci][0]:
                emit_sx_issue(ci)
                for qq in range(*chunks[ci]):
                    passB_sched.append((cur_iter[0] + 2, qq))

        def sched_passA(q):
            lgt = wk.tile([128, 4096], BF16, tag="lgt", name=f"lgt{q}")
            ses = wk.tile([128, 8], F32, tag="ses", name=f"ses{q}")
            for cc in range(0, VL, 512):
                qA.append(lambda q=q, cc=cc, lgt=lgt, ses=ses:
                          emit_passA_chunk(q, cc, lgt, ses))
            qA.append(lambda q=q, lgt=lgt, ses=ses: emit_passA_fin(q, lgt, ses))

        def emit_sx_issue(ci):
            nc.sync.dma_start(sxins[ci][:], sumexp[:, chunks[ci][0]:chunks[ci][1]])
            if not no_collectives:
                nc.gpsimd.collective_compute(
                    "AllGather", ALU.bypass, replica_groups=rg,
                    ins=[sxins[ci][:].opt()], outs=[sxouts[ci][:].opt()])

        def emit_sx_fin(ci):
            a, b = chunks[ci]
            w = b - a
            ssum = wk.tile([128, 2 * TS], F32, tag="ssum")
            if not no_collectives:
                sgr = wk.tile([128, 8 * TS], F32, tag="sgr")
                nc.sync.dma_start(
                    sgr[:, 0:w * n_cores].rearrange("p (t c) -> p t c",
                                                    c=n_cores),
                    sxouts[ci][:].rearrange("c p t -> p t c"))
                nc.vector.reduce_sum(
                    out=ssum[:, 0:w],
                    in_=sgr[:, 0:w * n_cores].rearrange("p (t c) -> p t c",
                                                        c=n_cores),
                    axis=mybir.AxisListType.X)
            else:
                nc.vector.tensor_copy(ssum[:, 0:w], sumexp[:, a:b])
            nc.vector.reciprocal(ssum[:, TS:TS + w], ssum[:, 0:w])
            nc.scalar.activation(out=nlz[:, a:b], in_=ssum[:, TS:TS + w],
                                 func=AF.Ln)

        def emit_passB_load(q):
            lgB = wk.tile([128, VL], BF16, tag="lgB", name=f"lgB{q}", bufs=1)
            nc.sync.dma_start(lgB[:], lstage[q])
            stash[("B", q)] = lgB

        def emit_passB_fin(q, piece):
            lgB = stash[("B", q)] if piece == 0 else stash.pop(("B", q))
            cs = slice(piece * 2000, (piece + 1) * 2000)
            lp = wk.tile([128, 2000], F32, tag="lp")
            nc.gpsimd.tensor_scalar_add(lp[:], lgB[:, cs], nlz[:, q:q + 1])
            nc.sync.dma_start(out_lp.ap()[:, q, cs], lp[:])

        def sched_passB(q):
            qB.append(lambda q=q: emit_passB_load(q))
            qB.append(lambda q=q: emit_passB_fin(q, 0))
            qB.append(lambda q=q: emit_passB_fin(q, 1))

        # ---------------- schedule bookkeeping ----------------
        gather_emitted = [False] * len(chunks)
        passA_sched = []   # (ready_iter, q)
        passA_done_chunk = [0] * len(chunks)
        passB_sched = []   # (ready_iter, q)
        cur_iter = [0]

        def chunk_of(q):
            return next(i for i, (a, b) in enumerate(chunks) if a <= q < b)

        def on_step_end(t):
            for ci, (a, b) in enumerate(chunks):
                if t == b - 1 and not gather_emitted[ci]:
                    gather_emitted[ci] = True
                    if not no_collectives:
                        nc.gpsimd.collective_compute(
                            "AllGather", ALU.bypass, replica_groups=rg,
                            ins=[hist[a:b].opt()],
                            outs=[gats[ci][:].opt()])
                    else:
                        for c in range(n_cores):
                            nc.sync.dma_start(gats[ci][c], hist[a:b])
                    for q in range(a, b):
                        passA_sched.append((t + 2, q))

        def drain_schedules(budget_a, budget_b):
            na = 0
            while passA_sched and na < budget_a:
                if passA_sched[0][0] > cur_iter[0]:
                    break
                _, q = passA_sched.pop(0)
                qB.append(lambda q=q: emit_hT_load(q))
                sched_passA(q)
                na += 1
            nb = 0
            while passB_sched and nb < budget_b:
                if passB_sched[0][0] > cur_iter[0]:
                    break
                _, q = passB_sched.pop(0)
                ci = chunk_of(q)
                if q == chunks[ci][0]:
                    qB.append(lambda ci=ci: emit_sx_fin(ci))
                sched_passB(q)
                nb += 1

        # ---------------- recurrence (half-offset software pipeline) ----
        # Per iteration t: h1.combine(t-1) | h0.dense(t) | h0.pre(t) |
        # h1.dense(t) | h0.combine(t) | h1.pre(t).  Each half's dense work
        # fills the other half's serial softmax/GRU chain.
        hidFs = [None, None]
        hid8s = [None, None]
        for h in range(NH):
            hidFs[h] = hidp.tile([128, HK, BH], F32, tag=f"hidF{h}",
                                 name=f"hidF{h}")
            nc.sync.dma_start(
                hidFs[h][:], hid0T.ap().rearrange("k p b -> p k b")
                [:, :, h * BH:(h + 1) * BH])
            hid8s[h] = hidp.tile([128, HK, BH], FP8, tag=f"hid8{h}",
                                 name=f"hid8{h}")
            nc.sync.dma_start(
                hid8s[h][:], hid0T8.ap().rearrange("k p b -> p k b")
                [:, :, h * BH:(h + 1) * BH])

        a3s = [None, None]
        expes = [None, None]
        ctx8s = [None, None]

        def dense(h, t):
            # a1 = tanh((W1h@hid + b1 + encproj64)/64), then d2, d3
            a1 = ap_.tile([128, HK, BH * L], FP8, tag=f"a1_{h}",
                          name=f"a1_{h}", bufs=1)
            php = phps[h]
            for m in range(HK):
                o = php[:, m, :]
                for kp in range(0, HK, 2):
                    _mm(nc, o, w1hT_sb[:, kp:kp + 2, m * 128:(m + 1) * 128],
                        hid8s[h][:, kp:kp + 2, :], kp == 0, False, pm=DR)
                _mm(nc, o, b1T_sb[:, m * 128:(m + 1) * 128],
                    onesb[0:1, 0:BH], False, True)
            a1p = wk.tile([128, HK, BH * L], BF16, tag=f"a1p{h}",
                          name=f"a1p{h}", bufs=1)
            for m in range(HK):
                pslc = php[:, m, :]
                phb = AP(tensor=pslc.tensor, offset=pslc.offset,
                         ap=pslc.ap + [[0, L]])
                nc.vector.tensor_add(
                    a1p[:, m, :].rearrange("p (b l) -> p b l", b=BH),
                    phb, encprojT[:, m, h * BH:(h + 1) * BH, :])
                nc.scalar.activation(out=a1[:, m, :], in_=a1p[:, m, :],
                                     func=AF.Tanh, scale=RS)
            pump(qA, 1)
            src_ = a1
            for (wT, bT, dtag) in ((w2T_sb, b2T_sb, "a2"), (w3T_sb, b3T_sb, "a3")):
                dst = ap_.tile([128, HK, BH * L], FP8, tag=f"{dtag}_{h}",
                               name=f"{dtag}_{h}", bufs=1)
                for m in range(HK):
                    pdt = pd.tile([128, 512], F32, tag="pd")
                    for kp in range(0, HK, 2):
                        _mm(nc, pdt[:],
                            wT[:, kp:kp + 2, m * 128:(m + 1) * 128],
                            src_[:, kp:kp + 2, :], kp == 0, False, pm=DR)
                    _mm(nc, pdt[:], bT[:, m * 128:(m + 1) * 128],
                        onesb[0:1, 0:512], False, True)
                    nc.scalar.activation(out=dst[:, m, :], in_=pdt[:],
                                         func=AF.Tanh, scale=RS)
                src_ = dst
                pump(qA, 1)
            a3s[h] = src_

        def pre(h, t):
            # e-dot, exp, strips, ctx, normalize, gates matmuls
            a3 = a3s[h]
            pe = pe_of(h)
            for kk in range(RH):
                for kp in range(0, HK, 2):
                    _mm(nc, pe[:, kk:kk + 1],
                        a3[:, kp:kp + 2, kk * 128:(kk + 1) * 128],
                        vT_sb[:, kp:kp + 2, :], kp == 0, kp == 2, pm=DR)
            expe = wk.tile([128, RH], F32, tag=f"expe{h}", name=f"expe{h}")
            nc.scalar.activation(out=expe[:], in_=pe[:], func=AF.Exp, scale=RS)
            lo = masks[h][0:64]
            lo = AP(tensor=lo.tensor, offset=lo.offset,
                    ap=[lo.ap[0], [BH + 2, RH]])
            nc.vector.tensor_copy(lo, expe[0:64, :])
            hi = masks[h][64:128, :, 1:2]
            hi = AP(tensor=hi.tensor, offset=hi.offset,
                    ap=[hi.ap[0], [BH + 2, RH]])
            nc.vector.tensor_copy(hi, expe[64:128, :])
            pcu = pd.tile([128, 512], F32, tag="pd", name=f"pcu{h}")
            for kk in range(RH):
                _mm(nc, pcu[h * 32:h * 32 + BH, :], masks[h][:, kk, :],
                    enc_sb[:, RH * h + kk, :], kk == 0, kk == RH - 1)
                _mm(nc, zs[h][h * 32:h * 32 + BH, :], masks[h][:, kk, :],
                    onesc[:, 0:1], kk == 0, kk == RH - 1)
            ps = slice(h * 32, h * 32 + BH)
            cu = wk.tile([64, H], BF16, tag=f"cu{h}", name=f"cu{h}")
            rcz = wk.tile([64, 1], F32, tag=f"rcz{h}", name=f"rcz{h}")
            diag = wk.tile([64, BH], BF16, tag=f"diag{h}", name=f"diag{h}")
            nc.vector.reciprocal(rcz[ps, :], zs[h][ps, :])
            nc.vector.tensor_scalar_mul(
                diag[ps, :], ident[ps, h * 32:h * 32 + BH], rcz[ps, :])
            nc.vector.tensor_copy(cu[ps, :], pcu[ps, :])
            ctxT = ctxT_of(h)
            for m in range(HK):
                _mm(nc, ctxT[:, m, :], cu[ps, m * 128:(m + 1) * 128],
                    diag[ps, :], True, True)
            ctx8 = wk.tile([128, HK, BH], FP8, tag=f"c8{h}", name=f"c8{h}")
            nc.vector.tensor_copy(ctx8[:], ctxT[:])
            P1, P2 = pgts[h][0], pgts[h][1]
            for c in range(12):
                o1 = P1[:, c, :]
                for kp in range(0, HK, 2):
                    _mm(nc, o1, whhT_sb[:, kp:kp + 2, c * 128:(c + 1) * 128],
                        hid8s[h][:, kp:kp + 2, :], kp == 0, False, pm=DR)
                _mm(nc, o1, bhhT_sb[:, c * 128:(c + 1) * 128],
                    onesb[0:1, 0:BH], False, True)
            for c in range(12):
                o2 = P2[:, c, :]
                for kp in range(0, HK, 2):
                    _mm(nc, o2, wihcT_sb[:, kp:kp + 2, c * 128:(c + 1) * 128],
                        ctx8[:, kp:kp + 2, :], kp == 0, False, pm=DR)
                _mm(nc, o2, bihT_sb[:, c * 128:(c + 1) * 128],
                    onesb[0:1, 0:BH], False, True)

        def combine(h, t):
            # gate combine (tanh-only sigmoid), writes new hid + hist half
            po = (t % 8) * BL
            tc_ = t // 8
            P1 = pgts[h][0][:]
            gie = giT_emb[:, :, tc_, po + h * BH:po + (h + 1) * BH]
            g2 = wk.tile([128, 12, BH], F32, tag=f"g2{h}", name=f"g2{h}")
            nc.vector.tensor_copy(g2[:], pgts[h][1][:])
            P2 = g2[:]
            rzp = wk.tile([128, 8, BH], F32, tag=f"rzp{h}", name=f"rzp{h}")
            nc.vector.tensor_add(rzp[:], P1[:, 0:8, :], P2[:, 0:8, :])
            nc.vector.tensor_add(rzp[:], rzp[:], gie[:, 0:8, :])
            th = wk.tile([128, 8, BH], F32, tag=f"th{h}", name=f"th{h}")
            nc.scalar.activation(out=th[:], in_=rzp[:], func=AF.Tanh,
                                 scale=1.0 / 128.0)
            s2 = wk.tile([128, HK, BH], F32, tag=f"s2{h}", name=f"s2{h}")
            nc.vector.scalar_tensor_tensor(
                out=s2[:], in0=P1[:, 8:12, :], scalar=0.5,
                in1=P2[:, 8:12, :], op0=ALU.mult, op1=ALU.add)
            nc.vector.tensor_add(s2[:], s2[:], gie[:, 8:12, :])
            t1 = wk.tile([128, HK, BH], F32, tag=f"t1{h}", name=f"t1{h}")
            nc.vector.scalar_tensor_tensor(
                out=t1[:], in0=P1[:, 8:12, :], scalar=1.0 / 128.0,
                in1=th[:, 0:4, :], op0=ALU.mult, op1=ALU.mult)
            nc.vector.scalar_tensor_tensor(
                out=s2[:], in0=s2[:], scalar=RS, in1=t1[:],
                op0=ALU.mult, op1=ALU.add)
            thn = wk.tile([128, HK, BH], F32, tag=f"thn{h}", name=f"thn{h}")
            nc.scalar.activation(out=thn[:], in_=s2[:], func=AF.Tanh)
            d = wk.tile([128, HK, BH], F32, tag=f"d{h}", name=f"d{h}")
            nc.vector.tensor_sub(d[:], hidFs[h][:], thn[:])
            e2 = wk.tile([128, HK, BH], F32, tag=f"e2{h}", name=f"e2{h}")
            nc.vector.tensor_mul(e2[:], th[:, 4:8, :], d[:])
            nc.vector.tensor_add(e2[:], e2[:], d[:])
            hidF_new = hidp.tile([128, HK, BH], F32, tag=f"hidF{h}",
                                 name=f"hidF{h}")
            hid8_new = hidp.tile([128, HK, BH], FP8, tag=f"hid8{h}",
                                 name=f"hid8{h}")
            nc.vector.scalar_tensor_tensor(
                out=hidF_new[:], in0=e2[:], scalar=0.5,
                in1=thn[:], op0=ALU.mult, op1=ALU.add)
            nc.vector.tensor_copy(hid8_new[:], hidF_new[:])
            hidFs[h], hid8s[h] = hidF_new, hid8_new
            nc.sync.dma_start(
                hist[t].rearrange("k p b -> p k b")[:, :, h * BH:(h + 1) * BH],
                hid8_new[:])

        for t in range(t_steps):
            cur_iter[0] = t
            drain_schedules(2, 2)
            pump(qB, 4)
            if t > 0:
                combine(1, t - 1)
                on_step_end(t - 1)
            dense(0, t)
            pump(qA, 1)
            pre(0, t)
            pump(qA, 2)
            dense(1, t)
            pump(qA, 1)
            combine(0, t)
            pre(1, t)
            pump(qA, 2)
            pump(qB, 3)
        combine(1, t_steps - 1)
        on_step_end(t_steps - 1)

        # ---------------- tail drain ----------------
        while passA_sched or passB_sched or qA or qB:
            cur_iter[0] += 1
            drain_schedules(3, 3)
            pump(qB, 4)
            pump(qA, 6)

    nc.compile()
    return nc


_NC_CACHE = {}


def _get_program(t_steps=TS, n_cores=NC, **kw):
    key = (t_steps, n_cores, tuple(sorted(kw.items())))
    if key not in _NC_CACHE:
        _NC_CACHE[key] = build_program(t_steps, n_cores, **kw)
    return _NC_CACHE[key]


def make_in_maps(inputs, t_steps=TS, n_cores=NC):
    """Host-side shard/layout prep. Pure data movement + dtype casts."""
    bf = ml_dtypes.bfloat16
    f8 = ml_dtypes.float8_e4m3
    enc = np.asarray(inputs["encoder_outputs"], np.float32)
    ehid = np.asarray(inputs["encoder_hidden"], np.float32)
    targets = np.asarray(inputs["targets"])
    embW = np.ascontiguousarray(np.asarray(inputs["embed_W"], np.float32))
    aW1 = np.asarray(inputs["att_W1"], np.float32)
    aW2 = np.asarray(inputs["att_W2"], np.float32)
    aW3 = np.asarray(inputs["att_W3"], np.float32)
    ab1 = np.asarray(inputs["att_b1"], np.float32)
    ab2 = np.asarray(inputs["att_b2"], np.float32)
    ab3 = np.asarray(inputs["att_b3"], np.float32)
    av = np.asarray(inputs["att_v"], np.float32)
    gWih = np.asarray(inputs["gru_Wih"], np.float32)
    gWhh = np.asarray(inputs["gru_Whh"], np.float32)
    gbih = np.asarray(inputs["gru_bih"], np.float32)
    gbhh = np.asarray(inputs["gru_bhh"], np.float32)
    oW = np.asarray(inputs["out_W"], np.float32)
    ob = np.asarray(inputs["out_b"], np.float32)

    def chunkT(w, dt, scale=1.0):   # (out,in)->(in,out) chunked [HK,128,out]
        wt = np.ascontiguousarray((w.T * scale).astype(dt))
        return wt.reshape(HK, 128, w.shape[0])

    shared = {
        "embW": embW,
        "w1eT_in": chunkT(aW1[:, :H], bf),
        "w1hT_in": chunkT(aW1[:, H:], f8, WS),
        "w2T_in": chunkT(aW2, f8, WS),
        "w3T_in": chunkT(aW3, f8, WS),
        "vT_in": np.ascontiguousarray((av[0] * WS).astype(f8)).reshape(HK, 128, 1),
        "b1T_in": (ab1 * WS).astype(bf).reshape(1, H),
        "b2T_in": (ab2 * WS).astype(bf).reshape(1, H),
        "b3T_in": (ab3 * WS).astype(bf).reshape(1, H),
        "wiheT_in": chunkT(gWih[:, :WORD], bf, WS),
        "wihcT_in": chunkT(gWih[:, WORD:], f8, WS),
        "whhT_in": chunkT(gWhh, f8, WS),
        "bihT_in": (gbih * WS).astype(bf).reshape(1, G3),
        "bhhT_in": (gbhh * WS).astype(bf).reshape(1, G3),
    }
    in_maps = []
    for c in range(n_cores):
        bl0 = c * BL
        enc_l = enc[bl0:bl0 + BL].reshape(NR, H)
        idx = np.zeros(512, np.int32)
        idx[: BL * t_steps] = targets[bl0:bl0 + BL, :t_steps].T.astype(np.int32).ravel()
        h0 = ehid[0, bl0:bl0 + BL]                    # [BL, H]
        m = dict(shared)
        m["enc_in"] = np.ascontiguousarray(enc_l.astype(bf)).reshape(RK, 128, H)
        m["encT_in"] = np.ascontiguousarray(enc_l.T.astype(bf)).reshape(HK, 128, NR)
        m["hid0T"] = np.ascontiguousarray(h0.T).reshape(HK, 128, BL)
        m["hid0T8"] = np.ascontiguousarray(h0.T.astype(f8)).reshape(HK, 128, BL)
        m["tgt_idx"] = idx.reshape(4, 128, 1)
        m["outWT_in"] = np.ascontiguousarray(
            (oW[c * VL:(c + 1) * VL].T * WS).astype(f8)).reshape(HK, 128, VL)
        m["outb_in"] = (ob[c * VL:(c + 1) * VL] * WS).astype(bf).reshape(1, VL)
        in_maps.append(m)
    return in_maps


def run(inputs, trace=False, **trace_kw):
    from concourse import bass_utils
    nc = _get_program()
    in_maps = make_in_maps(inputs)
    res = bass_utils.run_bass_kernel_spmd(nc, in_maps, core_ids=list(range(NC)),
                                          trace=trace, **trace_kw)
    out = np.concatenate([res.results[c]["out_lp"] for c in range(NC)], axis=2)
    return out, res


def kernel(**inputs):
    return run(inputs)[0]
